# revision 1
# baseline (speedup 1.0000x reference)
"""Bidirectional LSTM (S=2048, B=4096, I=1, H=8, O=1) on 8 Trainium2 NeuronCores.

Strategy
--------
Pure data parallel over batch (512 rows/core) + sequence chunking with warmup:
the forget-gate contraction (~0.35/step) makes a chunk started W=9 steps
early from zero state converge to the exact trajectory below fp16 noise.

Per core: NP=7 (fwd,bwd) chunk-stream pairs x G=2 half-phase pipelined groups
(28 chunks of l=147 total).  Streams stack block-diagonally in the matmul:
rhs = [h (112 rows) ; x (14) ; ones (1)] = [127, 512] fp16.  Per group-step:

  PE : 4 fp16 matmuls (f, i, o -> one [112,3,512] PSUM tile; g -> [119,512]).
       The g stationary carries 7 extra columns computing the output
       projection 0.5*(w_out.h + b_out) and gate biases ride the ones-row,
       so out-proj + all biases cost zero extra instructions.
  ACT: ONE sigmoid over the merged f,i,o tile; tanh over the g tile (its
       rows 112:119 become tanh(0.5*(out+b_out)) - transported through the
       existing tanh and decoded host-side as 2*arctanh, so output
       evacuation needs no copy at all); tanh(c).
  DVE: tm=f*c, z=i*g, c'=tm+z, h'=o*tanh(c') - 4 fp16 tensor_tensor ops
       (2x DVE perf mode).
  DMA: next x rows into the next rhs tile (SP queue); fp16-encoded out rows
       straight from the tanh-output SBUF tile to HBM (Pool queue).

PSUM: per group one [112,3,512] f32 tile (f,i,o) + one [119,512] (g) =
4 banks; 2 groups = all 8 banks.  Emission order back(g1,r-1), front(g0,r),
front(g1,r), back(g0,r) keeps the ready-to-run tanh(c)/h' of the trailing
group at the head of the ACT/DVE queues (no head-of-line blocking).

Warmup uses stationary copies with pair-0 gate columns zeroed so chunk 0
starts exactly from zero state.  A final flush round (R = l+W+1) emits the
last position's output, since out(pos p) rides the matmul of round p+W+1.

Measured: ~1.10 ms HW exec across 8 cores (baseline 1.26 ms), rel err 4.0e-3
(fp16 datapath; fp32 reference tolerance 2e-2).
"""

import os
import sys

if "axon" not in os.environ.get("JAX_PLATFORMS", "axon"):
    os.environ["JAX_PLATFORMS"] = "axon,cpu"

try:
    import concourse  # noqa: F401
except ImportError:  # pragma: no cover
    sys.path.insert(0, "/opt/trn_rl_repo")

from contextlib import ExitStack

import numpy as np

import concourse.bacc as bacc
import concourse.mybir as mybir
import concourse.tile as tile

S, B, I, H, O = 2048, 4096, 1, 8, 1
N_CORES = 8
BC = B // N_CORES  # batch columns per core

NP = 7      # stream pairs per group
G = 2       # pipelined groups per core
W = 9       # warmup rounds per chunk

KH = 16 * NP           # h rows / gate partitions (112)
KR = KH + 2 * NP + 1   # rhs rows: h + x + ones (127)
KO = KH + NP           # o-gate dst partitions incl. out rows (119)

GATES = ("g", "f", "i", "o")
TORCH_BLOCK = {"i": 0, "f": 1, "g": 2, "o": 3}  # torch LSTM gate row blocks

F32 = mybir.dt.float32
F16 = mybir.dt.float16
AF = mybir.ActivationFunctionType


def _lchunk(s_len, n_pairs, n_groups):
    n_chunks = n_pairs * n_groups
    return -(-s_len // n_chunks)  # ceil; tail chunk padded with zero x


# --------------------------------------------------------------------------
# host-side data preparation
# --------------------------------------------------------------------------

def make_weights(wihs, whhs, bihs, bhhs, w_out, b_out):
    """Block-diagonal fp16 stationaries [KR, M] per gate (+ warm variants).

    Rows 0..KH: h rows; KH..KH+2NP: x rows; last row: ones (bias row).
    g-gate has M=KO: cols KH..KH+NP are the fused out-projection, scaled by
    0.5 and decoded host-side with 2*arctanh (b_out baked into the ones-row).
    """
    out = {}
    for t in GATES:
        bi = TORCH_BLOCK[t]
        M = KO if t == "g" else KH
        w = np.zeros((KR, M), np.float32)
        for s in range(NP):
            for d in range(2):
                c0 = 16 * s + 8 * d
                w[c0:c0 + 8, c0:c0 + 8] = whhs[d][8 * bi:8 * bi + 8, :].T
                w[KH + 2 * s + d, c0:c0 + 8] = wihs[d][8 * bi:8 * bi + 8, 0]
                w[KR - 1, c0:c0 + 8] = (bihs[d] + bhhs[d])[8 * bi:8 * bi + 8]
        if t == "g":
            # fused out-projection: rides the g matmul + its tanh ACT.
            # Scaled by 0.5 so tanh stays near-linear; host: 2*arctanh.
            for s in range(NP):
                w[16 * s:16 * s + 8, KH + s] = 0.5 * w_out[0, 0:8]
                w[16 * s + 8:16 * s + 16, KH + s] = 0.5 * w_out[0, 8:16]
                w[KR - 1, KH + s] = 0.5 * b_out
        w_warm = w.copy()
        w_warm[:, 0:16] = 0.0  # keep pair-0 (h,c) identically 0 during warmup
        out[f"w_{t}"] = w.astype(np.float16)
        out[f"w_{t}_warm"] = w_warm.astype(np.float16)
    return out


def make_xarr(x_core, future, l_chunk):
    """Per-core x arranged as [G, R, 2*NP+1, BC] fp16; last row is ones."""
    s_len, bc = x_core.shape
    R = l_chunk + W + 1
    xb = x_core[(future - np.arange(s_len)) % s_len]
    xarr = np.zeros((G, R, 2 * NP + 1, bc), np.float32)
    xarr[:, :, 2 * NP, :] = 1.0
    rr = np.arange(R)
    for g in range(G):
        for s in range(NP):
            pos = (g * NP + s) * l_chunk - W + rr
            valid = (pos >= 0) & (pos < s_len)
            for d, src in enumerate((x_core, xb)):
                xarr[g, valid, 2 * s + d, :] = src[pos[valid]]
    return xarr.astype(np.float16)


def make_in_maps(x, wihs, whhs, bihs, bhhs, w_out, b_out, future,
                 use_f32r=None):
    shared = make_weights(wihs, whhs, bihs, bhhs, w_out, float(b_out))
    l_chunk = _lchunk(S, NP, G)
    in_maps = []
    for k in range(N_CORES):
        m = dict(shared)
        m["xarr"] = make_xarr(x[:, k * BC:(k + 1) * BC, 0], future, l_chunk)
        in_maps.append(m)
    return in_maps


# --------------------------------------------------------------------------
# program builder
# --------------------------------------------------------------------------

def build_program(bc=BC, s_len=S, num_devices=N_CORES):
    l_chunk = _lchunk(s_len, NP, G)
    s_pad = l_chunk * NP * G
    R = l_chunk + W + 1

    nc = bacc.Bacc("TRN2", target_bir_lowering=False, debug=False,
                   enable_asserts=False, num_devices=num_devices)

    dram = {}
    host_names = []

    def din(name, shape):
        dram[name] = nc.dram_tensor(name, list(shape), F16, kind="ExternalInput").ap()
        host_names.append(name)

    for t in GATES:
        M = KO if t == "g" else KH
        din(f"w_{t}", (KR, M))
        din(f"w_{t}_warm", (KR, M))
    din("xarr", (G, R, 2 * NP + 1, bc))
    out_d = nc.dram_tensor("out", [s_pad, bc], F16, kind="ExternalOutput").ap()
    out_view = out_d.rearrange("(c l) b -> c l b", l=l_chunk)

    with tile.TileContext(nc) as tc, ExitStack() as ctx:
        consts = ctx.enter_context(tc.tile_pool(name="consts", bufs=1))
        hp = ctx.enter_context(tc.tile_pool(name="hp", bufs=4))
        osb3 = ctx.enter_context(tc.tile_pool(name="osb3", bufs=4))
        cp = ctx.enter_context(tc.tile_pool(name="cp", bufs=3))
        up = ctx.enter_context(tc.tile_pool(name="up", bufs=3))
        zp = ctx.enter_context(tc.tile_pool(name="zp", bufs=3))
        # osb allocates no tiles but its SBUF reservation shifts the layout;
        # removing it reproducibly costs ~220us (1107us -> 1322us), most
        # likely via SBUF bank conflicts between engine streams.  KEEP.
        osb = ctx.enter_context(tc.tile_pool(name="osb", bufs=4))
        osb2 = ctx.enter_context(tc.tile_pool(name="osb2", bufs=4))
        fps = ctx.enter_context(tc.tile_pool(name="fps", bufs=1, space="PSUM"))
        gps = ctx.enter_context(tc.tile_pool(name="gps", bufs=1, space="PSUM"))

        ct = {}
        for name, ap in dram.items():
            if name == "xarr":
                continue
            t_ = consts.tile(list(ap.shape), ap.dtype, name=f"c_{name}", tag=f"c_{name}")
            nc.sync.dma_start(out=t_, in_=ap)
            ct[name] = t_

        rhs_cur, c_prev = [], []
        fio_ps, g_ps = [], []
        for g in range(G):
            r0t = hp.tile([KR, bc], F16, name=f"rhs0_{g}", tag=f"h{g}")
            nc.vector.memset(r0t[0:KH, :], 0.0)
            nc.sync.dma_start(out=r0t[KH:KR, :], in_=dram["xarr"][g, 0])
            c0 = cp.tile([KH, bc], F16, name=f"c0_{g}", tag=f"c{g}")
            nc.vector.memset(c0, 0.0)
            rhs_cur.append(r0t)
            c_prev.append(c0)
            fio_ps.append(fps.tile([KH, 3, bc], F32, name=f"fio_{g}", tag=f"fio{g}"))
            g_ps.append(gps.tile([KO, bc], F32, name=f"gps_{g}", tag=f"g{g}"))

        u_fio = [None] * G
        u_g = [None] * G

        def front(g, r):
            """matmuls f,i,o,g + sigmoid(f,i,o) + tanh(g|out) + cell DVE."""
            warm = "_warm" if (g == 0 and r < W) else ""
            rhs = rhs_cur[g]
            if r == R - 1:  # flush round: only the out columns matter
                nc.tensor.matmul(g_ps[g], ct["w_g"], rhs, start=True, stop=True)
                u_g[g] = up.tile([KO, bc], F16, name=f"ug_{g}_{r}", tag=f"ug{g}")
                nc.scalar.activation(u_g[g], g_ps[g], AF.Tanh)
                return
            nc.tensor.matmul(fio_ps[g][:, 0, :], ct[f"w_f{warm}"], rhs,
                             start=True, stop=True)
            nc.tensor.matmul(fio_ps[g][:, 1, :], ct[f"w_i{warm}"], rhs,
                             start=True, stop=True)
            nc.tensor.matmul(fio_ps[g][:, 2, :], ct[f"w_o{warm}"], rhs,
                             start=True, stop=True)
            nc.tensor.matmul(g_ps[g], ct[f"w_g{warm}"], rhs, start=True, stop=True)
            u_fio[g] = up.tile([KH, 3, bc], F16, name=f"uf_{g}_{r}", tag=f"uf{g}")
            nc.scalar.activation(u_fio[g], fio_ps[g], AF.Sigmoid)
            u_g[g] = up.tile([KO, bc], F16, name=f"ug_{g}_{r}", tag=f"ug{g}")
            nc.scalar.activation(u_g[g], g_ps[g], AF.Tanh)
            tm = zp.tile([KH, bc], F16, name=f"t_{g}_{r}", tag=f"tm{g}")
            nc.vector.tensor_mul(tm, u_fio[g][:, 0, :], c_prev[g])
            z = zp.tile([KH, bc], F16, name=f"z_{g}_{r}", tag=f"z{g}")
            nc.vector.tensor_mul(z, u_fio[g][:, 1, :], u_g[g][0:KH, :])
            cn = cp.tile([KH, bc], F16, name=f"c_{g}_{r}", tag=f"c{g}")
            nc.vector.tensor_add(cn, tm, z)
            c_prev[g] = cn

        def back(g, r):
            """tanh(c) + h' + x DMA + out DMA (tanh-encoded, from u_g rows)."""
            if r + 1 < R:
                rhs_n = hp.tile([KR, bc], F16, name=f"rhs_{g}_{r}", tag=f"h{g}")
                nc.sync.dma_start(out=rhs_n[KH:KR, :], in_=dram["xarr"][g, r + 1])
                th = zp.tile([KH, bc], F16, name=f"th_{g}_{r}", tag=f"th{g}")
                nc.scalar.activation(th, c_prev[g], AF.Tanh)
                nc.vector.tensor_mul(rhs_n[0:KH, :], u_fio[g][:, 2, :], th)
                rhs_cur[g] = rhs_n
            if r >= W + 1:
                # u_g rows KH:KO hold tanh(0.5*(w_out.h_{r-1} + b_out))
                nc.gpsimd.dma_start(
                    out=out_view[g * NP:(g + 1) * NP, r - 1 - W, :],
                    in_=u_g[g][KH:KO, :])

        for r in range(R):
            if r > 0:
                back(1, r - 1)
            front(0, r)
            front(1, r)
            back(0, r)
        back(1, R - 1)

    nc.compile()
    return nc, host_names


# --------------------------------------------------------------------------
# runner
# --------------------------------------------------------------------------

_CACHE = {}


def _get_program(use_f32r=None):
    key = (NP, G, W, BC, S)
    if key not in _CACHE:
        _CACHE[key] = build_program()
    return _CACHE[key]


def kernel(x, w_ih_f, w_hh_f, b_ih_f, b_hh_f, w_ih_b, w_hh_b, b_ih_b, b_hh_b,
           w_out, b_out, future):
    from concourse import bass_utils

    x = np.asarray(x, np.float32)
    wihs = [np.asarray(w_ih_f, np.float32), np.asarray(w_ih_b, np.float32)]
    whhs = [np.asarray(w_hh_f, np.float32), np.asarray(w_hh_b, np.float32)]
    bihs = [np.asarray(b_ih_f, np.float32), np.asarray(b_ih_b, np.float32)]
    bhhs = [np.asarray(b_hh_f, np.float32), np.asarray(b_hh_b, np.float32)]
    w_out = np.asarray(w_out, np.float32)
    b_out = float(np.asarray(b_out).reshape(-1)[0])
    future = int(future)

    nc, names = _get_program()
    in_maps = make_in_maps(x, wihs, whhs, bihs, bhhs, w_out, b_out, future)
    res = bass_utils.run_bass_kernel_spmd(nc, in_maps, core_ids=list(range(N_CORES)))
    out = np.empty((B, S), np.float32)
    for k in range(N_CORES):
        y = np.asarray(res.results[k]["out"][:S, :], np.float32)
        out[k * BC:(k + 1) * BC, :] = 2.0 * np.arctanh(y).T
    return out

